# revision 1
# baseline (speedup 1.0000x reference)
"""Multi-head attention (ReLU-gated projections) on 8 Trainium2 NeuronCores.

Problem (hardcoded): B=4, S=1024, H=1024, NH=16, DH=64.
  qp = relu(q @ Wq.T + bq); kp, vp likewise
  alpha = softmax(qh @ kh.T / sqrt(DH)) * mask[q]
  out = (alpha @ vh).reshape(B,S,H) + query

Sharding: 8 cores = 4 batches x 2 head-groups (8 heads / 512 hidden cols each).

Per-core device kernel (all in transposed "hidden-on-partitions" layout):
  stage 1: qpT[o,s], kpT[o,s] (transposed) and vp[s,o] (normal) projections
           with fused bias+relu. Contraction over h via PE; inputs fed
           host-pre-transposed (xT = x.T per batch).
  stage 2: per head: alphaT[k,q] = khT.T @ qhT (K=64); P=exp(alpha/8) on ACT
           (no max subtraction needed: alpha/8 <= ~5); AV via PE with a ones
           column appended to v so row 64 of the output accumulates
           sumexp[q] for free.  Output: unnormalized hidT (64,S) + sumexp (S)
           per head; host divides, applies mask, adds residual.
"""
import sys

sys.path.insert(0, "/opt/trn_rl_repo")

import os
import numpy as np
import ml_dtypes

import concourse.bass as bass
import concourse.tile as tile
from concourse import bacc, mybir
from concourse import bass_utils

if os.environ.get("BASS_LDW_OPT", "0") == "1":
    _orig_run_command = bass_utils.run_command

    def _patched_run_command(cmd, **kw):
        cmd = ["--enable-ldw-opt=true" if c == "--enable-ldw-opt=false" else c
               for c in cmd]
        return _orig_run_command(cmd, **kw)

    bass_utils.run_command = _patched_run_command

B, S, H = 4, 1024, 1024
NH, DH = 16, 64
NCORES = 8
GROUPS = 2          # head-groups (tensor-parallel dim)
HL = NH // GROUPS   # heads per core = 8
GH = H // GROUPS    # hidden cols per core = 512
KT = H // 128       # contraction k-tiles = 8
OT = GH // 128      # output o-tiles per core = 4
SCALE = 1.0 / float(np.sqrt(DH))

# matmul precision mode: "f32" (exact, 4 cyc/row), "f32r" (TF32-ish, 1 cyc/row),
# "bf16" (1 cyc/row, smallest footprint)
MODE = os.environ.get("BASS_MM_DT", "bf16")
ALPHA_ILV = os.environ.get("BASS_ALPHA_ILV", "1") == "1"

F32 = mybir.dt.float32
F32R = mybir.dt.float32r
BF16 = mybir.dt.bfloat16


def _cfg(mode):
    if mode == "bf16":
        return dict(np_dt=ml_dtypes.bfloat16, io_dt=BF16, st_dt=BF16,
                    cast=False, pt_bufs=36, hid_bufs=3, x_bufs=16,
                    shift_alphas=True, kz=True)
    if mode == "f32r":
        # float32r end-to-end: walrus requires f32r matmul inputs to be
        # *produced* as f32r (DMA loads + DVE/ACT evacuations), not bitcast.
        return dict(np_dt=np.float32, io_dt=F32R, st_dt=F32R,
                    cast=False, pt_bufs=9, hid_bufs=2, x_bufs=8,
                    shift_alphas=False, kz=False)
    return dict(np_dt=np.float32, io_dt=F32, st_dt=F32,
                cast=False, pt_bufs=9, hid_bufs=2, x_bufs=8,
                shift_alphas=False, kz=False)


def _mm(ap, cast):
    return ap.bitcast(F32R) if cast else ap


def build(mode):
    cfg = _cfg(mode)
    io_dt, st_dt, cast = cfg["io_dt"], cfg["st_dt"], cfg["cast"]
    nc = bacc.Bacc("TRN2", target_bir_lowering=False, debug=False,
                   num_devices=NCORES)

    xq_d = nc.dram_tensor("xq", [H, S], io_dt, kind="ExternalInput").ap()
    xk_d = nc.dram_tensor("xk", [H, S], io_dt, kind="ExternalInput").ap()
    xv_d = nc.dram_tensor("xv", [H, S], io_dt, kind="ExternalInput").ap()
    wq_d = nc.dram_tensor("wq", [H, GH], io_dt, kind="ExternalInput").ap()
    wk_d = nc.dram_tensor("wk", [H, GH], io_dt, kind="ExternalInput").ap()
    wv_d = nc.dram_tensor("wv", [H, GH], io_dt, kind="ExternalInput").ap()
    bqk_d = nc.dram_tensor("bqk", [128, 2 * OT], F32, kind="ExternalInput").ap()
    bv_d = nc.dram_tensor("bv", [1, GH], io_dt, kind="ExternalInput").ap()
    ones_d = nc.dram_tensor("onesd", [128, 128], io_dt,
                            kind="ExternalInput").ap()
    zeros_d = nc.dram_tensor("zerosd", [64, S], io_dt,
                             kind="ExternalInput").ap()
    hid_d = nc.dram_tensor("hid", [HL * (DH + 1), S], F32,
                           kind="ExternalOutput").ap()

    with tile.TileContext(nc) as tc:
        with tc.tile_pool(name="sb", bufs=1) as sb, \
             tc.tile_pool(name="ps", bufs=1, space="PSUM") as ps:

            full_x = mode == "bf16"   # x resident for full S vs per-chunk

            # ---- persistent tiles; one big DMA per tensor (>=1MB, descriptor
            #      runs of 1-2KB/partition), spread across the three DGE rings
            #      (sync / scalar / gpsimd) so loads overlap ----
            wq_t = sb.tile([128, KT * GH], io_dt, tag="wq", name="wq")
            wk_t = sb.tile([128, KT * GH], io_dt, tag="wk", name="wk")
            wv_t = sb.tile([128, KT * GH], io_dt, tag="wv", name="wv")
            qp_t = [sb.tile([128, S], st_dt, tag=f"qp{t}", name=f"qp{t}")
                    for t in range(OT)]
            KZ = cfg["kz"]
            if KZ:
                # zero-padded K copies: kz[t][h] holds head h's kh rows in its
                # own 64-partition half, zeros in the other -> full-K=128
                # alpha matmuls whose weight loads pipeline like any other MM
                kz_t = [[sb.tile([128, S], st_dt, tag=f"kz{t}{h}",
                                 name=f"kz{t}{h}") for h in range(2)]
                        for t in range(OT)]
                kz_zeroed = set()
            else:
                kp_t = [sb.tile([128, S], st_dt, tag=f"kp{t}",
                                name=f"kp{t}") for t in range(OT)]
            # v laid out [k-tile x head x (64 v cols + ones col)]
            VW = HL * (DH + 1)
            vp_t = sb.tile([128, KT * VW], st_dt, tag="vp", name="vp")
            bqk_t = sb.tile([128, 2 * OT], F32, tag="bqk", name="bqk")
            bv_t = sb.tile([1, GH], io_dt, tag="bv", name="bv")
            ones_t = sb.tile([1, 128], io_dt, tag="ones", name="ones")

            def w_ld(w_t, w_d, eng):
                # SBUF [128, k*GH + o]  <-  DRAM [(k p) o]
                eng.dma_start(w_t[:].rearrange("p (k o) -> p k o", o=GH),
                              w_d.rearrange("(k p) o -> p k o", p=128))

            x_t = {}

            def x_ld(which, x_d, eng, sc, eng2=None):
                # one chunk (512 s-cols) of one input, laid [128, k*512+s];
                # optionally split across two DGE rings for latency
                tag = f"x{which}{sc}" if full_x else f"x{which}"
                t = sb.tile([128, KT * 512], io_dt, tag=tag,
                            name=f"x{which}_{sc}")
                src3 = x_d.rearrange("(k p) s -> p k s",
                                     p=128)[:, :, sc * 512:(sc + 1) * 512]
                dst3 = t[:].rearrange("p (k s) -> p k s", s=512)
                if eng2 is None:
                    eng.dma_start(dst3, src3)
                else:
                    h = KT // 2
                    eng.dma_start(dst3[:, :h], src3[:, :h])
                    eng2.dma_start(dst3[:, h:], src3[:, h:])
                x_t[(which, sc)] = t

            def xap(which, sc, k):
                return x_t[(which, sc)][:, k * 512:(k + 1) * 512]

            nc.sync.dma_start(bv_t[:], bv_d)
            nc.sync.dma_start(ones_t[:], ones_d[0:1, :])
            nc.sync.dma_start(bqk_t[:], bqk_d)
            ones64_t = sb.tile([128, KT * HL], io_dt, tag="ones64",
                               name="ones64")
            nc.sync.dma_start(ones64_t[:], ones_d[:, 0:KT * HL])
            x_ld("q", xq_d, nc.scalar, 0)
            w_ld(wq_t, wq_d, nc.sync)
            x_ld("k", xk_d, nc.sync, 0)
            w_ld(wk_t, wk_d, nc.scalar)
            x_ld("v", xv_d, nc.scalar, 0)
            w_ld(wv_t, wv_d, nc.sync)

            # HAM warmup: ~8us of tiny matmuls on early-arriving const tiles
            # so the real matmuls start at 2.4GHz instead of 1.2
            warm = ps.tile([128, 512], F32, tag="av", bufs=2, name="warm")
            for i in range(22):
                nc.tensor.matmul(warm[:], _mm(ones_t[:], cast),
                                 _mm(bv_t[:], cast), start=True, stop=True)
            # preload the ACT exp table set (~2.7us) while ACT is still idle
            dummy_exp = sb.tile([1, 8], F32, tag="dummy_exp", name="dummy_exp")
            nc.scalar.activation(dummy_exp[:], ones_t[0:1, 0:8],
                                 mybir.ActivationFunctionType.Exp, scale=1.0)
            if full_x:
                x_ld("q", xq_d, nc.sync, 1)
                x_ld("k", xk_d, nc.sync, 1)
                x_ld("v", xv_d, nc.gpsimd, 1)
            v4 = vp_t[:].rearrange("p (k n c) -> p k n c", n=HL, c=DH + 1)
            nc.vector.tensor_copy(
                v4[:, :, :, DH:DH + 1],
                ones64_t[:].rearrange("p (k n one) -> p k n one", n=HL, one=1))

            def proj_qk(sc, ot, only=None):
                """one o-tile, one s-chunk of the transposed q/k projections"""
                for which, w_t, xw in (("q", wq_t, "q"), ("k", wk_t, "k")):
                    if only is not None and which != only:
                        continue
                    pp = ps.tile([128, 1024], F32, tag="alpha", bufs=3,
                                 name=f"pp{which}_{sc}_{ot}")
                    for k in range(KT):
                        nc.tensor.matmul(
                            pp[:, 0:512],
                            _mm(w_t[:, k * GH + ot * 128:
                                    k * GH + (ot + 1) * 128], cast),
                            _mm(xap(xw, sc, k), cast),
                            start=(k == 0), stop=(k == KT - 1))
                    wi = 0 if which == "q" else 1
                    bias = bqk_t[:, wi * OT + ot:wi * OT + ot + 1]
                    ssl = slice(sc * 512, (sc + 1) * 512)

                    def evac(dst, pslice, bias_ap):
                        # chunk-0 evacs go to the (idle-at-that-point) ACT
                        # engine: relu(x*1 + bias); chunk-1 to DVE
                        if sc == 0 and KZ:
                            nc.scalar.activation(
                                dst, pslice,
                                mybir.ActivationFunctionType.Relu,
                                bias=bias_ap, scale=1.0)
                        else:
                            nc.vector.tensor_scalar(
                                dst, pslice, bias_ap, 0.0,
                                mybir.AluOpType.add, mybir.AluOpType.max)

                    if which == "q":
                        evac(qp_t[ot][:, ssl], pp[:, 0:512], bias)
                    elif KZ:
                        for h in range(2):
                            pr = slice(h * 64, h * 64 + 64)
                            evac(kz_t[ot][h][pr, ssl], pp[pr, 0:512],
                                 bias[pr, :])
                    else:
                        nc.vector.tensor_scalar(
                            kp_t[ot][:, ssl], pp[:, 0:512], bias, 0.0,
                            mybir.AluOpType.add, mybir.AluOpType.max)

            def proj_v(sc, j):
                """one s-tile (128 rows of vp) within chunk sc"""
                st = sc * 4 + j
                pp = ps.tile([128, 1024], F32, tag="alpha", bufs=3,
                             name=f"ppv_{st}")
                nc.tensor.matmul(pp[:, 0:512], _mm(ones_t[:], cast),
                                 _mm(bv_t[:], cast), start=True, stop=False)
                for k in range(KT):
                    nc.tensor.matmul(
                        pp[:, 0:512],
                        _mm(xap("v", sc, k)[:, j * 128:(j + 1) * 128], cast),
                        _mm(wv_t[:, k * GH:(k + 1) * GH], cast),
                        start=False, stop=(k == KT - 1))
                v3 = vp_t[:, st * VW:(st + 1) * VW].rearrange(
                    "p (n c) -> p n c", c=DH + 1)
                p3 = pp[:, 0:512].rearrange("p (n c) -> p n c", c=DH)
                nc.vector.tensor_scalar(
                    v3[:, :, 0:DH], p3, 0.0, None, mybir.AluOpType.max)

            pt_all = {}

            def alphas(n0):
                """alpha + exp for head pair (n0, n0+1); the two heads live on
                disjoint 64-partition halves of o-tile n0//2, so adjacent
                matmuls target disjoint PE row-groups and overlap."""
                t = n0 // 2
                if t not in kz_zeroed:
                    kz_zeroed.add(t)
                    nc.vector.memset(kz_t[t][0][64:128, :], 0.0)
                    nc.vector.memset(kz_t[t][1][0:64, :], 0.0)
                pts0, pts1 = [], []
                for k in range(KT):
                    apts = []
                    for h in range(2):
                        apt = ps.tile([128, 1024], F32, tag="alpha", bufs=3,
                                      name=f"alp_{n0 + h}_{k}")
                        apts.append(apt)
                    for qc in range(2):
                        for h in range(2):
                            nc.tensor.matmul(
                                apts[h][:, qc * 512:(qc + 1) * 512],
                                _mm(kz_t[t][h][:, k * 128:(k + 1) * 128],
                                    cast),
                                _mm(qp_t[t][:, qc * 512:(qc + 1) * 512],
                                    cast),
                                start=True, stop=True)
                    for h, pts in ((0, pts0), (1, pts1)):
                        pt = sb.tile([128, 1024], st_dt, tag="pt",
                                     bufs=cfg["pt_bufs"], name=f"pt_{n0 + h}_{k}")
                        nc.scalar.activation(pt[:], apts[h][:],
                                             mybir.ActivationFunctionType.Exp,
                                             scale=SCALE)
                        pts.append(pt)
                pt_all[n0] = pts0
                pt_all[n0 + 1] = pts1

            def head_seq(n):
                """unpaired alpha+exp then AV for one head (low pt_bufs modes)"""
                t, off = n // 2, (n % 2) * 64
                pts = []
                for k in range(KT):
                    apt = ps.tile([128, 1024], F32, tag="alpha", bufs=3,
                                  name=f"alp_{n}_{k}")
                    for qc in range(2):
                        nc.tensor.matmul(
                            apt[:, qc * 512:(qc + 1) * 512],
                            _mm(kp_t[t][off:off + 64,
                                        k * 128:(k + 1) * 128], cast),
                            _mm(qp_t[t][off:off + 64,
                                        qc * 512:(qc + 1) * 512], cast),
                            start=True, stop=True)
                    pt = sb.tile([128, 1024], st_dt, tag="pt",
                                 bufs=cfg["pt_bufs"], name=f"pt_{n}_{k}")
                    nc.scalar.activation(pt[:], apt[:],
                                         mybir.ActivationFunctionType.Exp,
                                         scale=SCALE)
                    pts.append(pt)
                pt_all[n] = pts

            def avs(n):
                pts = pt_all.pop(n)
                hid_t = sb.tile([DH + 1, S], F32, tag="hid",
                                bufs=cfg["hid_bufs"], name=f"hid_{n}")
                for qc in range(2):
                    av = ps.tile([DH + 1, 512], F32, tag="av", bufs=2,
                                 name=f"av_{n}_{qc}")
                    for k in range(KT):
                        nc.tensor.matmul(
                            av[:],
                            _mm(vp_t[:, k * VW + n * (DH + 1):
                                     k * VW + (n + 1) * (DH + 1)], cast),
                            _mm(pts[k][:, qc * 512:(qc + 1) * 512], cast),
                            start=(k == 0), stop=(k == KT - 1))
                    nc.vector.tensor_copy(
                        hid_t[:, qc * 512:(qc + 1) * 512], av[:])
                    nc.sync.dma_start(
                        hid_d[n * (DH + 1):(n + 1) * (DH + 1),
                              qc * 512:(qc + 1) * 512],
                        hid_t[:, qc * 512:(qc + 1) * 512])

            # ---- emission schedule ----
            if cfg["shift_alphas"]:
                for ot in range(OT):
                    proj_qk(0, ot, only="q")
                for ot in range(OT):
                    proj_qk(0, ot, only="k")
            else:
                for ot in range(OT):
                    proj_qk(0, ot)
            for j in range(4):
                proj_v(0, j)
            if not full_x:
                x_ld("q", xq_d, nc.sync, 1)
                x_ld("k", xk_d, nc.sync, 1)
                x_ld("v", xv_d, nc.gpsimd, 1)
            if cfg["shift_alphas"]:
                proj_qk(1, 0)
                alphas(0)
                for j in range(4):
                    proj_v(1, j)
                proj_qk(1, 1)
                alphas(2)
                avs(0)
                avs(1)
                proj_qk(1, 2)
                alphas(4)
                avs(2)
                avs(3)
                proj_qk(1, 3)
                alphas(6)
                avs(4)
                avs(5)
                avs(6)
                avs(7)
            else:
                proj_qk(1, 0)
                head_seq(0)
                for j in range(4):
                    proj_v(1, j)
                head_seq(1)
                avs(0)
                avs(1)
                for ot in range(1, OT):
                    proj_qk(1, ot)
                    head_seq(2 * ot)
                    avs(2 * ot)
                    head_seq(2 * ot + 1)
                    avs(2 * ot + 1)

    nc.compile()
    return nc


_NC_CACHE = {}


def _get_nc(mode):
    if mode not in _NC_CACHE:
        _NC_CACHE[mode] = build(mode)
    return _NC_CACHE[mode]


def _prep_inputs(inputs, mode):
    cfg = _cfg(mode)
    np_dt = cfg["np_dt"]
    q = np.asarray(inputs["query"], np.float32)
    k = np.asarray(inputs["key"], np.float32)
    v = np.asarray(inputs["value"], np.float32)
    Wq = np.asarray(inputs["Wq"], np.float32)
    Wk = np.asarray(inputs["Wk"], np.float32)
    Wv = np.asarray(inputs["Wv"], np.float32)
    bq = np.asarray(inputs["bq"], np.float32)
    bk = np.asarray(inputs["bk"], np.float32)
    bv = np.asarray(inputs["bv"], np.float32)

    xq = [np.ascontiguousarray(q[b].T).astype(np_dt) for b in range(B)]
    xk = [np.ascontiguousarray(k[b].T).astype(np_dt) for b in range(B)]
    xv = [np.ascontiguousarray(v[b].T).astype(np_dt) for b in range(B)]
    in_maps = []
    for c in range(NCORES):
        b, g = c // GROUPS, c % GROUPS
        sl = slice(g * GH, (g + 1) * GH)
        bqk = np.stack([bq[sl].reshape(OT, 128).T,
                        bk[sl].reshape(OT, 128).T], 1).reshape(128, 2 * OT)
        in_maps.append({
            "xq": xq[b], "xk": xk[b], "xv": xv[b],
            "wq": np.ascontiguousarray(Wq[sl, :].T).astype(np_dt),
            "wk": np.ascontiguousarray(Wk[sl, :].T).astype(np_dt),
            "wv": np.ascontiguousarray(Wv[sl, :].T).astype(np_dt),
            "bqk": np.ascontiguousarray(bqk, dtype=np.float32),
            "bv": np.ascontiguousarray(bv[None, sl]).astype(np_dt),
            "onesd": np.ones((128, 128), np_dt),
            "zerosd": np.zeros((64, S), np_dt),
        })
    return in_maps


def run(inputs, mode=MODE, trace=False):
    nc = _get_nc(mode)
    in_maps = _prep_inputs(inputs, mode)
    res = bass_utils.run_bass_kernel_spmd(
        nc, in_maps, core_ids=list(range(NCORES)), trace=trace)

    masks = np.asarray(inputs["masks"], np.float32)
    query = np.asarray(inputs["query"], np.float32)
    out = np.empty((B, S, H), np.float32)
    for c in range(NCORES):
        b, g = c // GROUPS, c % GROUPS
        hid = res.results[c]["hid"].reshape(HL, DH + 1, S)
        hT = hid[:, :DH, :]                      # (HL, DH, S)
        se = hid[:, DH, :]                       # (HL, S)
        blk = (hT / se[:, None, :]).transpose(2, 0, 1).reshape(S, GH)
        out[b, :, g * GH:(g + 1) * GH] = blk
    out = out * masks[:, :, None] + query
    return out, res


def kernel(**inputs) -> np.ndarray:
    out, _ = run(inputs)
    return out



# revision 10
# speedup vs baseline: 1.2465x; 1.2465x over previous
"""Multi-head attention (ReLU-gated projections) on 8 Trainium2 NeuronCores.

Problem (hardcoded): B=4, S=1024, H=1024, NH=16, DH=64.
  qp = relu(q @ Wq.T + bq); kp, vp likewise
  alpha = softmax(qh @ kh.T / sqrt(DH)) * mask[q]
  out = (alpha @ vh).reshape(B,S,H) + query

Sharding: 8 cores = 4 batches x 2 head-groups (8 heads / 512 hidden cols each).

fp8 design (per core):
  - inputs x/W quantized host-side to fp8e4m3 (TRN float8e4, max 240).
  - projections as fp8 DoubleRow matmuls (2x contraction per cycle):
    qp/kp evac'd with fused bias+relu to bf16, vp to fp8 (with a ones column
    per head so AV accumulates sumexp for free, plus one pad column so the
    DoubleRow pair stride is 16B-aligned: 66 cols/head).
  - alpha: bf16 K=64 matmuls, two heads concurrently on disjoint 64-row
    PE row-groups (2x row tiling; tile_position auto-derived from
    base_partition of the kp/qp slices).
  - softmax exp with a global shift: pt = exp(alpha*SCALE - SHIFT) so pt fits
    fp8e4m3 range; numerator and denominator scale together so the softmax is
    exact.  exp split across two engines: ACT (spline exp -> fp8 out) and DVE
    (Schraudolph: k8 = (alpha*A + B) as int8, bitcast to fp8e4m3).
  - AV: fp8 DoubleRow (vp pairs of k-chunks stationary, pt pairs moving),
    av PSUM [66,512] DMA'd directly to DRAM (rows 0-64; row 64 = sumexp).
  - host divides by sumexp, applies mask, adds residual.
"""
import sys

sys.path.insert(0, "/opt/trn_rl_repo")

import math
import os
import numpy as np
import ml_dtypes

import concourse.bass as bass
import concourse.tile as tile
from concourse import bacc, mybir
from concourse import bass_utils

if os.environ.get("BASS_LDW_OPT", "0") == "1":
    _orig_run_command = bass_utils.run_command

    def _patched_run_command(cmd, **kw):
        cmd = ["--enable-ldw-opt=true" if c == "--enable-ldw-opt=false" else c
               for c in cmd]
        return _orig_run_command(cmd, **kw)

    bass_utils.run_command = _patched_run_command

B, S, H = 4, 1024, 1024
NH, DH = 16, 64
NCORES = 8
GROUPS = 2          # head-groups (tensor-parallel dim)
HL = NH // GROUPS   # heads per core = 8
GH = H // GROUPS    # hidden cols per core = 512
KT = H // 128       # contraction k-tiles = 8
OT = GH // 128      # output o-tiles per core = 4
SCALE = 1.0 / float(np.sqrt(DH))
SHIFT = 4.0         # global exp shift: pt = exp(alpha*SCALE - SHIFT)
PVW = DH + 2        # padded per-head v width (64 v + 1 ones + 1 pad) = 66
VW = HL * PVW       # v cols per k-chunk = 528 (16B aligned)

# Schraudolph constants for fp8e4m3 (bias 7, 3 mantissa bits):
#   k8 = (alpha * SCALE - SHIFT) * 8/ln2 + 56 - c ; bitcast int8 -> fp8
_LN2 = math.log(2.0)
SCHR_C = float(os.environ.get("BASS_SCHR_C", "0.45"))
SCHR_A = SCALE * 8.0 / _LN2
# +0.5: DVE f32->int8 convert truncates (matches CoreSim); makes it rounding.
SCHR_B = 56.0 - SCHR_C - SHIFT * 8.0 / _LN2 \
    + float(os.environ.get("BASS_SCHR_HALF", "0.5"))

MODE = os.environ.get("BASS_MM_DT", "fp8")
WARM = int(os.environ.get("BASS_WARM", "12"))
ACT_EXPS = int(os.environ.get("BASS_ACT_EXPS", "33"))  # of 64 exp tiles

F32 = mybir.dt.float32
BF16 = mybir.dt.bfloat16
FP8 = mybir.dt.float8e4
I8 = mybir.dt.int8
DR = mybir.MatmulPerfMode.DoubleRow
E4NP = ml_dtypes.float8_e4m3


def build(mode):
    assert mode == "fp8"
    nc = bacc.Bacc("TRN2", target_bir_lowering=False, debug=False,
                   num_devices=NCORES)

    # x: [p, sc(2), k(8), s'(512)] -> [128, 8192]; h = k*128+p, s = sc*512+s'
    xq_d = nc.dram_tensor("xq", [128, 2 * KT * 512], FP8,
                          kind="ExternalInput").ap()
    xk_d = nc.dram_tensor("xk", [128, 2 * KT * 512], FP8,
                          kind="ExternalInput").ap()
    xv_d = nc.dram_tensor("xv", [128, 2 * KT * 512], FP8,
                          kind="ExternalInput").ap()
    # wq/wk: [p, ot(4), k(8), o'(128)] -> [128, 4096]
    wq_d = nc.dram_tensor("wq", [128, OT * KT * 128], FP8,
                          kind="ExternalInput").ap()
    wk_d = nc.dram_tensor("wk", [128, OT * KT * 128], FP8,
                          kind="ExternalInput").ap()
    # wv: [p, k(8), o(512)] -> [128, 4096]
    wv_d = nc.dram_tensor("wv", [128, KT * GH], FP8, kind="ExternalInput").ap()
    bqk_d = nc.dram_tensor("bqk", [128, 2 * OT], F32, kind="ExternalInput").ap()
    bv_d = nc.dram_tensor("bv", [1, GH], FP8, kind="ExternalInput").ap()
    hid_d = nc.dram_tensor("hid", [HL * (DH + 1), S], BF16,
                           kind="ExternalOutput").ap()

    with tile.TileContext(nc) as tc:
        with tc.tile_pool(name="sb", bufs=1) as sb, \
             tc.tile_pool(name="ps", bufs=1, space="PSUM") as ps:

            # ---- persistent SBUF tiles ----
            x_t = {}
            x_t[("q", 0)] = sb.tile([128, KT * 512], FP8, tag="xq0", name="xq0")
            x_t[("q", 1)] = sb.tile([128, KT * 512], FP8, tag="xq1", name="xq1")
            x_t[("k", 0)] = sb.tile([128, KT * 512], FP8, tag="xk0", name="xk0")
            x_t[("k", 1)] = sb.tile([128, KT * 512], FP8, tag="xk1", name="xk1")
            x_t[("v", 0)] = sb.tile([128, KT * 512], FP8, tag="xv0", name="xv0")
            x_t[("v", 1)] = sb.tile([128, KT * 512], FP8, tag="xv1", name="xv1")
            wq_t = sb.tile([128, OT * KT * 128], FP8, tag="wq", name="wq")
            wk_t = sb.tile([128, OT * KT * 128], FP8, tag="wk", name="wk")
            wv_t = sb.tile([128, KT * GH], FP8, tag="wv", name="wv")
            qp_t = [sb.tile([128, S], BF16, tag=f"qp{t}", name=f"qp{t}")
                    for t in range(OT)]
            kp_t = [sb.tile([128, S], BF16, tag=f"kp{t}", name=f"kp{t}")
                    for t in range(OT)]
            vp_t = sb.tile([128, KT * VW], FP8, tag="vp", name="vp")
            bqk_t = sb.tile([128, 2 * OT], F32, tag="bqk", name="bqk")
            bv_t = sb.tile([1, GH], FP8, tag="bv", name="bv")
            wa_t = sb.tile([1, 128], FP8, tag="wa", name="wa")
            wb_t = sb.tile([1, 512], FP8, tag="wb", name="wb")
            dummy_t = sb.tile([1, 8], F32, tag="dummy", name="dummy")
            nshift_t = sb.tile([128, 1], F32, tag="nshift", name="nshift")

            # ---- t=0: DMA-free warmup (memset inputs) + const setup ----
            nc.vector.memset(wa_t[:], 1.0)
            nc.vector.memset(wb_t[:], 0.0)
            nc.gpsimd.memset(nshift_t[:], -SHIFT)
            # vp ones + pad columns (per head, per k-chunk)
            vp4 = vp_t[:].rearrange("p (k n c) -> p k n c", n=HL, c=PVW)
            nc.vector.memset(vp4[:, :, :, DH:DH + 1], 1.0)
            nc.vector.memset(vp4[:, :, :, DH + 1:DH + 2], 0.0)
            for i in range(WARM):
                warm = ps.tile([128, 512], F32, tag="small", bufs=2,
                               name=f"warm{i}")
                nc.tensor.matmul(warm[:], wa_t[:], wb_t[:],
                                 start=True, stop=True)
            # preload ACT exp table while idle
            nc.scalar.activation(dummy_t[:], wb_t[0:1, 0:8],
                                 mybir.ActivationFunctionType.Exp, scale=1.0)

            # ---- input DMAs, priority-ordered across the three rings ----
            # gpsimd: biases + weights; sync: x chunk 0; scalar: x chunk 1
            nc.gpsimd.dma_start(bqk_t[:], bqk_d)
            nc.gpsimd.dma_start(bv_t[:], bv_d)
            nc.gpsimd.dma_start(wq_t[:, 0:2 * KT * 128],
                                wq_d[:, 0:2 * KT * 128])
            nc.gpsimd.dma_start(wk_t[:, 0:2 * KT * 128],
                                wk_d[:, 0:2 * KT * 128])
            nc.gpsimd.dma_start(wq_t[:, 2 * KT * 128:],
                                wq_d[:, 2 * KT * 128:])
            nc.gpsimd.dma_start(wk_t[:, 2 * KT * 128:],
                                wk_d[:, 2 * KT * 128:])
            nc.gpsimd.dma_start(wv_t[:], wv_d)
            for which, xd in (("q", xq_d), ("k", xk_d), ("v", xv_d)):
                nc.sync.dma_start(x_t[(which, 0)][:], xd[:, 0:KT * 512])
                nc.scalar.dma_start(x_t[(which, 1)][:], xd[:, KT * 512:])

            # rearranged views
            def wqk_ot(w_t, ot):
                return w_t[:, ot * KT * 128:(ot + 1) * KT * 128].rearrange(
                    "p (k o) -> p k o", o=128)

            def x3(which, sc):
                return x_t[(which, sc)][:].rearrange("p (k s) -> p k s", s=512)

            wv3 = wv_t[:].rearrange("p (k o) -> p k o", o=GH)
            vp3 = vp_t[:].rearrange("p (k m) -> p k m", m=VW)

            # ---- engine balancing for exp tiles ----
            exp_state = {"acc": 0}

            def exp_engine():
                exp_state["acc"] += ACT_EXPS
                if exp_state["acc"] >= 64:
                    exp_state["acc"] -= 64
                    return "act"
                return "dve"

            pt_tiles = {}

            def pt_tile(n, c):
                if (n, c) not in pt_tiles:
                    pt_tiles[(n, c)] = sb.tile([128, 2048], FP8, tag="pt",
                                               bufs=32, name=f"pt_{n}_{c}")
                return pt_tiles[(n, c)]

            # ---- stage helpers ----
            def proj_qk(which, ot):
                w_t = wq_t if which == "q" else wk_t
                dst = qp_t[ot] if which == "q" else kp_t[ot]
                wi = 0 if which == "q" else 1
                wv_ot = wqk_ot(w_t, ot)
                bias = bqk_t[:, wi * OT + ot:wi * OT + ot + 1]
                pp = ps.tile([128, 1024], F32, tag="apt", bufs=3,
                             name=f"pp{which}{ot}")
                for sc in range(2):
                    xm = x3(which, sc)
                    for c2 in range(KT // 2):
                        nc.tensor.matmul(
                            pp[:, sc * 512:(sc + 1) * 512],
                            wv_ot[:, 2 * c2:2 * c2 + 2, :],
                            xm[:, 2 * c2:2 * c2 + 2, :],
                            start=(c2 == 0), stop=(c2 == KT // 2 - 1),
                            perf_mode=DR)
                nc.scalar.activation(
                    dst[:], pp[:],
                    mybir.ActivationFunctionType.Relu,
                    bias=bias, scale=1.0)

            def proj_v(st):
                sc, j = st // 4, st % 4
                pp = ps.tile([128, 512], F32, tag="small", bufs=2,
                             name=f"ppv{st}")
                nc.tensor.matmul(pp[:], wa_t[:], bv_t[:],
                                 start=True, stop=False)
                xm = x3("v", sc)
                for c2 in range(KT // 2):
                    nc.tensor.matmul(
                        pp[:],
                        xm[:, 2 * c2:2 * c2 + 2, j * 128:(j + 1) * 128],
                        wv3[:, 2 * c2:2 * c2 + 2, :],
                        start=False, stop=(c2 == KT // 2 - 1),
                        perf_mode=DR)
                # evac with relu into the strided fp8 v layout (cols 0..63)
                vdst = vp4[:, st, :, 0:DH]
                psrc = pp[:].rearrange("p (n c) -> p n c", c=DH)
                nc.vector.tensor_scalar(
                    vdst, psrc, 0.0, None, mybir.AluOpType.max)

            def alpha_pair(t, k):
                """alpha + exp for heads (2t, 2t+1), sk-tile k: two K=64
                matmuls on disjoint PE row-groups run concurrently."""
                apts = []
                for h in range(2):
                    apt = ps.tile([128, 1024], F32, tag="apt", bufs=3,
                                  name=f"alp_{2 * t + h}_{k}")
                    apts.append(apt)
                for qc in range(2):
                    for h in range(2):
                        pr = slice(h * 64, h * 64 + 64)
                        nc.tensor.matmul(
                            apts[h][:, qc * 512:(qc + 1) * 512],
                            kp_t[t][pr, k * 128:(k + 1) * 128],
                            qp_t[t][pr, qc * 512:(qc + 1) * 512],
                            start=True, stop=True)
                for h in range(2):
                    n = 2 * t + h
                    pt = pt_tile(n, k // 2)
                    half = pt[:, (k % 2) * 1024:(k % 2) * 1024 + 1024]
                    if exp_engine() == "act":
                        nc.scalar.activation(
                            half, apts[h][:],
                            mybir.ActivationFunctionType.Exp,
                            bias=nshift_t[:], scale=SCALE)
                    else:
                        nc.vector.tensor_scalar(
                            half.bitcast(I8), apts[h][:],
                            SCHR_A, SCHR_B,
                            mybir.AluOpType.mult, mybir.AluOpType.add)

            av_state = {"i": 0}

            def avs(n):
                av = ps.tile([128, 1024], F32, tag="apt", bufs=3,
                             name=f"av_{n}")
                for qc in range(2):
                    for c2 in range(KT // 2):
                        pt = pt_tile(n, c2)
                        ptm = pt[:].rearrange("p (two q) -> p two q", two=2)
                        nc.tensor.matmul(
                            av[0:PVW, qc * 512:(qc + 1) * 512],
                            vp3[:, 2 * c2:2 * c2 + 2,
                                n * PVW:(n + 1) * PVW],
                            ptm[:, :, qc * 512:(qc + 1) * 512],
                            start=(c2 == 0), stop=(c2 == KT // 2 - 1),
                            perf_mode=DR)
                hs = sb.tile([DH + 1, S], BF16, tag="hid", bufs=3,
                             name=f"hid_{n}")
                i = av_state["i"]
                av_state["i"] += 1
                if i % 2 == 0:
                    nc.scalar.copy(hs[:], av[0:DH + 1, :])
                else:
                    nc.vector.tensor_copy(hs[:], av[0:DH + 1, :])
                eng = nc.sync if i % 2 == 0 else nc.gpsimd
                eng.dma_start(hid_d[n * (DH + 1):(n + 1) * (DH + 1), :],
                              hs[:])

            # ---- emission schedule ----
            proj_qk("q", 0)
            proj_qk("k", 0)
            proj_qk("q", 1)
            proj_qk("k", 1)
            alpha_pair(0, 0)
            alpha_pair(0, 1)
            proj_v(0)
            alpha_pair(0, 2)
            alpha_pair(0, 3)
            proj_v(1)
            alpha_pair(0, 4)
            alpha_pair(0, 5)
            proj_v(2)
            alpha_pair(0, 6)
            alpha_pair(0, 7)
            proj_v(3)
            proj_qk("q", 2)
            proj_qk("k", 2)
            for k in range(KT):
                alpha_pair(1, k)
                if k % 2 == 1:
                    proj_v(4 + k // 2)
            proj_qk("q", 3)
            proj_qk("k", 3)
            for k in range(KT):
                alpha_pair(2, k)
            avs(0)
            avs(1)
            for k in range(KT):
                alpha_pair(3, k)
            avs(2)
            avs(3)
            avs(4)
            avs(5)
            avs(6)
            avs(7)

    nc.compile()
    return nc


_NC_CACHE = {}


def _get_nc(mode):
    if mode not in _NC_CACHE:
        _NC_CACHE[mode] = build(mode)
    return _NC_CACHE[mode]


def _prep_inputs(inputs, mode):
    q = np.asarray(inputs["query"], np.float32)
    k = np.asarray(inputs["key"], np.float32)
    v = np.asarray(inputs["value"], np.float32)
    Wq = np.asarray(inputs["Wq"], np.float32)
    Wk = np.asarray(inputs["Wk"], np.float32)
    Wv = np.asarray(inputs["Wv"], np.float32)
    bq = np.asarray(inputs["bq"], np.float32)
    bk = np.asarray(inputs["bk"], np.float32)
    bv = np.asarray(inputs["bv"], np.float32)

    def xprep(x, b):
        # [H, S] -> [p, sc, k, s'] -> [128, 8192] fp8
        xt = np.ascontiguousarray(x[b].T).astype(E4NP)
        return np.ascontiguousarray(
            xt.reshape(KT, 128, 2, 512).transpose(1, 2, 0, 3)
        ).reshape(128, 2 * KT * 512)

    def wqk_prep(W, sl):
        # W[sl].T: [H, GH] -> [p, ot, k, o'] -> [128, 4096] fp8
        wt = np.ascontiguousarray(W[sl, :].T).astype(E4NP)
        return np.ascontiguousarray(
            wt.reshape(KT, 128, OT, 128).transpose(1, 2, 0, 3)
        ).reshape(128, OT * KT * 128)

    def wv_prep(W, sl):
        # W[sl].T: [H, GH] -> [p, k, o] -> [128, 4096] fp8
        wt = np.ascontiguousarray(W[sl, :].T).astype(E4NP)
        return np.ascontiguousarray(
            wt.reshape(KT, 128, GH).transpose(1, 0, 2)
        ).reshape(128, KT * GH)

    xq = [xprep(q, b) for b in range(B)]
    xk = [xprep(k, b) for b in range(B)]
    xv = [xprep(v, b) for b in range(B)]
    in_maps = []
    for c in range(NCORES):
        b, g = c // GROUPS, c % GROUPS
        sl = slice(g * GH, (g + 1) * GH)
        bqk = np.stack([bq[sl].reshape(OT, 128).T,
                        bk[sl].reshape(OT, 128).T], 1).reshape(128, 2 * OT)
        in_maps.append({
            "xq": xq[b], "xk": xk[b], "xv": xv[b],
            "wq": wqk_prep(Wq, sl),
            "wk": wqk_prep(Wk, sl),
            "wv": wv_prep(Wv, sl),
            "bqk": np.ascontiguousarray(bqk, dtype=np.float32),
            "bv": np.ascontiguousarray(bv[None, sl]).astype(E4NP),
        })
    return in_maps


def run(inputs, mode=MODE, trace=False):
    nc = _get_nc(mode)
    in_maps = _prep_inputs(inputs, mode)
    res = bass_utils.run_bass_kernel_spmd(
        nc, in_maps, core_ids=list(range(NCORES)), trace=trace)

    masks = np.asarray(inputs["masks"], np.float32)
    query = np.asarray(inputs["query"], np.float32)
    out = np.empty((B, S, H), np.float32)
    for c in range(NCORES):
        b, g = c // GROUPS, c % GROUPS
        hid = np.asarray(res.results[c]["hid"],
                         dtype=np.float32).reshape(HL, DH + 1, S)
        hT = hid[:, :DH, :]                      # (HL, DH, S)
        se = hid[:, DH, :]                       # (HL, S)
        blk = (hT / se[:, None, :]).transpose(2, 0, 1).reshape(S, GH)
        out[b, :, g * GH:(g + 1) * GH] = blk
    out = out * masks[:, :, None] + query
    return out, res


def kernel(**inputs) -> np.ndarray:
    out, _ = run(inputs)
    return out


# revision 19
# speedup vs baseline: 1.4473x; 1.1611x over previous
"""Multi-head attention (ReLU-gated projections) on 8 Trainium2 NeuronCores.

Problem (hardcoded): B=4, S=1024, H=1024, NH=16, DH=64.
  qp = relu(q @ Wq.T + bq); kp, vp likewise
  alpha = softmax(qh @ kh.T / sqrt(DH)) * mask[q]
  out = (alpha @ vh).reshape(B,S,H) + query

Sharding: 8 cores = 4 batches x 2 head-groups (8 heads / 512 hidden cols each).

fp8 design (per core):
  - inputs x/W quantized host-side to fp8e4m3 (TRN float8e4, max 240).
  - projections as fp8 DoubleRow matmuls (2x contraction per cycle):
    qp/kp evac'd with fused bias+relu to bf16, vp to fp8 (with a ones column
    per head so AV accumulates sumexp for free, plus one pad column so the
    DoubleRow pair stride is 16B-aligned: 66 cols/head).
  - alpha: bf16 K=64 matmuls, two heads concurrently on disjoint 64-row
    PE row-groups (2x row tiling; tile_position auto-derived from
    base_partition of the kp/qp slices).
  - softmax exp with a global shift: pt = exp(alpha*SCALE - SHIFT) so pt fits
    fp8e4m3 range; numerator and denominator scale together so the softmax is
    exact.  exp split across two engines: ACT (spline exp -> fp8 out) and DVE
    (Schraudolph: k8 = (alpha*A + B) as int8, bitcast to fp8e4m3).
  - AV: fp8 DoubleRow (vp pairs of k-chunks stationary, pt pairs moving),
    av PSUM [66,512] DMA'd directly to DRAM (rows 0-64; row 64 = sumexp).
  - host divides by sumexp, applies mask, adds residual.
"""
import sys

sys.path.insert(0, "/opt/trn_rl_repo")

import math
import os
import numpy as np
import ml_dtypes

import concourse.bass as bass
import concourse.tile as tile
from concourse import bacc, mybir
from concourse import bass_utils

if os.environ.get("BASS_LDW_OPT", "0") == "1":
    _orig_run_command = bass_utils.run_command

    def _patched_run_command(cmd, **kw):
        cmd = ["--enable-ldw-opt=true" if c == "--enable-ldw-opt=false" else c
               for c in cmd]
        return _orig_run_command(cmd, **kw)

    bass_utils.run_command = _patched_run_command

B, S, H = 4, 1024, 1024
NH, DH = 16, 64
NCORES = 8
GROUPS = 2          # head-groups (tensor-parallel dim)
HL = NH // GROUPS   # heads per core = 8
GH = H // GROUPS    # hidden cols per core = 512
KT = H // 128       # contraction k-tiles = 8
OT = GH // 128      # output o-tiles per core = 4
SCALE = 1.0 / float(np.sqrt(DH))
SHIFT = 4.0         # global exp shift: pt = exp(alpha*SCALE - SHIFT)
PVW = DH + 2        # padded per-head v width (64 v + 1 ones + 1 pad) = 66
VW = HL * PVW       # v cols per k-chunk = 528 (16B aligned)

# Schraudolph constants for fp8e4m3 (bias 7, 3 mantissa bits):
#   k8 = (alpha * SCALE - SHIFT) * 8/ln2 + 56 - c ; bitcast int8 -> fp8
_LN2 = math.log(2.0)
SCHR_C = float(os.environ.get("BASS_SCHR_C", "0.45"))
SCHR_A = SCALE * 8.0 / _LN2
# +0.5: DVE f32->int8 convert truncates (matches CoreSim); makes it rounding.
SCHR_B = 56.0 - SCHR_C - SHIFT * 8.0 / _LN2 \
    + float(os.environ.get("BASS_SCHR_HALF", "0.5"))

MODE = os.environ.get("BASS_MM_DT", "fp8")
WARM = int(os.environ.get("BASS_WARM", "6"))
ACT_EXPS = int(os.environ.get("BASS_ACT_EXPS", "33"))  # of 64 exp tiles

F32 = mybir.dt.float32
BF16 = mybir.dt.bfloat16
FP8 = mybir.dt.float8e4
I8 = mybir.dt.int8
DR = mybir.MatmulPerfMode.DoubleRow
E4NP = ml_dtypes.float8_e4m3


def build(mode, bias_v=False):
    assert mode == "fp8"
    nc = bacc.Bacc("TRN2", target_bir_lowering=False, debug=False,
                   num_devices=NCORES)

    # x: [p, sc(2), k(8), s'(512)] -> [128, 8192]; h = k*128+p, s = sc*512+s'
    xq_d = nc.dram_tensor("xq", [128, 2 * KT * 512], FP8,
                          kind="ExternalInput").ap()
    xk_d = nc.dram_tensor("xk", [128, 2 * KT * 512], FP8,
                          kind="ExternalInput").ap()
    xv_d = nc.dram_tensor("xv", [128, 2 * KT * 512], FP8,
                          kind="ExternalInput").ap()
    # wq/wk: [p, ot(4), k(8), o'(128)] -> [128, 4096]
    wq_d = nc.dram_tensor("wq", [128, OT * KT * 128], FP8,
                          kind="ExternalInput").ap()
    wk_d = nc.dram_tensor("wk", [128, OT * KT * 128], FP8,
                          kind="ExternalInput").ap()
    # wv: [p, k(8), o(512)] -> [128, 4096]
    wv_d = nc.dram_tensor("wv", [128, KT * GH], FP8, kind="ExternalInput").ap()
    bqk_d = nc.dram_tensor("bqk", [128, 2 * OT], F32, kind="ExternalInput").ap()
    bv_d = nc.dram_tensor("bv", [1, GH], FP8, kind="ExternalInput").ap()
    hid_d = nc.dram_tensor("hid", [HL * (DH + 1), S], BF16,
                           kind="ExternalOutput").ap()

    with tile.TileContext(nc) as tc:
        with tc.tile_pool(name="sb", bufs=1) as sb, \
             tc.tile_pool(name="ps", bufs=1, space="PSUM") as ps:

            # ---- persistent SBUF tiles ----
            x_t = {}
            x_t[("q", 0)] = sb.tile([128, KT * 512], FP8, tag="xq0", name="xq0")
            x_t[("q", 1)] = sb.tile([128, KT * 512], FP8, tag="xq1", name="xq1")
            x_t[("k", 0)] = sb.tile([128, KT * 512], FP8, tag="xk0", name="xk0")
            x_t[("k", 1)] = sb.tile([128, KT * 512], FP8, tag="xk1", name="xk1")
            x_t[("v", 0)] = sb.tile([128, KT * 512], FP8, tag="xv0", name="xv0")
            x_t[("v", 1)] = sb.tile([128, KT * 512], FP8, tag="xv1", name="xv1")
            wq_t = sb.tile([128, OT * KT * 128], FP8, tag="wq", name="wq")
            wk_t = sb.tile([128, OT * KT * 128], FP8, tag="wk", name="wk")
            wv_t = sb.tile([128, KT * GH], FP8, tag="wv", name="wv")
            qp_t = [sb.tile([128, S], BF16, tag=f"qp{t}", name=f"qp{t}")
                    for t in range(OT)]
            kp_t = [sb.tile([128, S], BF16, tag=f"kp{t}", name=f"kp{t}")
                    for t in range(OT)]
            vp_t = sb.tile([128, KT * VW], FP8, tag="vp", name="vp")
            bqk_t = sb.tile([128, 2 * OT], F32, tag="bqk", name="bqk")
            bv_t = sb.tile([1, GH], FP8, tag="bv", name="bv")
            wa_t = sb.tile([128, 128], FP8, tag="wa", name="wa")
            wb_t = sb.tile([128, 512], FP8, tag="wb", name="wb")
            dummy_t = sb.tile([1, 8], F32, tag="dummy", name="dummy")
            nshift_t = sb.tile([128, 1], F32, tag="nshift", name="nshift")

            # ---- t=0: DMA-free warmup (memset inputs) + const setup ----
            nc.vector.memset(wa_t[:], 1.0)
            nc.vector.memset(wb_t[:], 0.0)
            nc.gpsimd.memset(nshift_t[:], -SHIFT)
            # vp ones + pad columns (per head, per k-chunk)
            vp4 = vp_t[:].rearrange("p (k n c) -> p k n c", n=HL, c=PVW)
            nc.vector.memset(vp4[:, :, :, DH:DH + 1], 1.0)
            nc.vector.memset(vp4[:, :, :, DH + 1:DH + 2], 0.0)
            for i in range(WARM):
                warm = ps.tile([128, 512], F32, tag="small", bufs=2,
                               name=f"warm{i}")
                nc.tensor.matmul(warm[:], wa_t[:], wb_t[:],
                                 start=True, stop=True)

            # ---- input DMAs: k-half quarters, priority q > k > v ----
            HK = KT // 2 * 512  # 2048 cols per (sc, k-half)

            def xq4(which, sc, kh):
                t = x_t[(which, sc)]
                xd = {"q": xq_d, "k": xk_d, "v": xv_d}[which]
                dst = t[:, kh * HK:(kh + 1) * HK]
                src = xd[:, sc * KT * 512 + kh * HK:
                         sc * KT * 512 + (kh + 1) * HK]
                return dst, src

            nc.gpsimd.dma_start(bqk_t[:], bqk_d)
            nc.gpsimd.dma_start(bv_t[:], bv_d)
            nc.gpsimd.dma_start(wq_t[:, 0:2 * KT * 128],
                                wq_d[:, 0:2 * KT * 128])
            nc.gpsimd.dma_start(wk_t[:, 0:2 * KT * 128],
                                wk_d[:, 0:2 * KT * 128])
            nc.gpsimd.dma_start(wq_t[:, 2 * KT * 128:],
                                wq_d[:, 2 * KT * 128:])
            nc.gpsimd.dma_start(wk_t[:, 2 * KT * 128:],
                                wk_d[:, 2 * KT * 128:])
            for which in ("q", "k"):
                for sc in (0, 1):
                    nc.sync.dma_start(*xq4(which, sc, 0))
                    nc.scalar.dma_start(*xq4(which, sc, 1))
            nc.sync.dma_start(*xq4("v", 0, 0))
            nc.scalar.dma_start(*xq4("v", 1, 0))
            nc.gpsimd.dma_start(*xq4("v", 0, 1))
            nc.gpsimd.dma_start(*xq4("v", 1, 1))
            nc.sync.dma_start(wv_t[:, 0:KT // 2 * GH], wv_d[:, 0:KT // 2 * GH])
            nc.scalar.dma_start(wv_t[:, KT // 2 * GH:], wv_d[:, KT // 2 * GH:])
            # preload ACT exp table (after the scalar-ring DMA kicks)
            nc.scalar.activation(dummy_t[:], wb_t[0:1, 0:8],
                                 mybir.ActivationFunctionType.Exp, scale=1.0)

            # rearranged views
            def wqk_ot(w_t, ot):
                return w_t[:, ot * KT * 128:(ot + 1) * KT * 128].rearrange(
                    "p (k o) -> p k o", o=128)

            def x3(which, sc):
                return x_t[(which, sc)][:].rearrange("p (k s) -> p k s", s=512)

            wv3 = wv_t[:].rearrange("p (k o) -> p k o", o=GH)
            vp3 = vp_t[:].rearrange("p (k m) -> p k m", m=VW)

            # ---- engine balancing for exp tiles ----
            exp_state = {"acc": 0}

            def exp_engine():
                exp_state["acc"] += ACT_EXPS
                if exp_state["acc"] >= 64:
                    exp_state["acc"] -= 64
                    return "act"
                return "dve"

            pt_tiles = {}

            def pt_tile(n, c):
                if (n, c) not in pt_tiles:
                    pt_tiles[(n, c)] = sb.tile([128, 2048], FP8, tag="pt",
                                               bufs=32, name=f"pt_{n}_{c}")
                return pt_tiles[(n, c)]

            # ---- stage helpers ----
            def proj_qk(which, ot):
                w_t = wq_t if which == "q" else wk_t
                dst = qp_t[ot] if which == "q" else kp_t[ot]
                wi = 0 if which == "q" else 1
                wv_ot = wqk_ot(w_t, ot)
                bias = bqk_t[:, wi * OT + ot:wi * OT + ot + 1]
                pp = ps.tile([128, 1024], F32, tag="apt", bufs=3,
                             name=f"pp{which}{ot}")
                for sc in range(2):
                    xm = x3(which, sc)
                    for c2 in range(KT // 2):
                        nc.tensor.matmul(
                            pp[:, sc * 512:(sc + 1) * 512],
                            wv_ot[:, 2 * c2:2 * c2 + 2, :],
                            xm[:, 2 * c2:2 * c2 + 2, :],
                            start=(c2 == 0), stop=(c2 == KT // 2 - 1),
                            perf_mode=DR)
                nc.scalar.activation(
                    dst[:], pp[:],
                    mybir.ActivationFunctionType.Relu,
                    bias=bias, scale=1.0)

            def proj_v(st):
                sc, j = st // 4, st % 4
                pp = ps.tile([128, 512], F32, tag="small", bufs=2,
                             name=f"ppv{st}")
                if bias_v:
                    nc.tensor.matmul(pp[:], wa_t[0:1, :], bv_t[:],
                                     start=True, stop=False)
                xm = x3("v", sc)
                for c2 in range(KT // 2):
                    nc.tensor.matmul(
                        pp[:],
                        xm[:, 2 * c2:2 * c2 + 2, j * 128:(j + 1) * 128],
                        wv3[:, 2 * c2:2 * c2 + 2, :],
                        start=(c2 == 0 and not bias_v),
                        stop=(c2 == KT // 2 - 1),
                        perf_mode=DR)
                # evac with relu into the strided fp8 v layout (cols 0..63)
                vdst = vp4[:, st, :, 0:DH]
                psrc = pp[:].rearrange("p (n c) -> p n c", c=DH)
                nc.vector.tensor_scalar(
                    vdst, psrc, 0.0, None, mybir.AluOpType.max)

            def alpha_pair(t, k):
                """alpha + exp for heads (2t, 2t+1), sk-tile k: two K=64
                matmuls on disjoint PE row-groups run concurrently."""
                apts = []
                for h in range(2):
                    apt = ps.tile([128, 1024], F32, tag="apt", bufs=3,
                                  name=f"alp_{2 * t + h}_{k}")
                    apts.append(apt)
                for h in range(2):
                    for qc in range(2):
                        pr = slice(h * 64, h * 64 + 64)
                        nc.tensor.matmul(
                            apts[h][:, qc * 512:(qc + 1) * 512],
                            kp_t[t][pr, k * 128:(k + 1) * 128],
                            qp_t[t][pr, qc * 512:(qc + 1) * 512],
                            start=True, stop=True)
                for h in range(2):
                    n = 2 * t + h
                    pt = pt_tile(n, k // 2)
                    half = pt[:, (k % 2) * 1024:(k % 2) * 1024 + 1024]
                    if exp_engine() == "act":
                        nc.scalar.activation(
                            half, apts[h][:],
                            mybir.ActivationFunctionType.Exp,
                            bias=nshift_t[:], scale=SCALE)
                    else:
                        nc.vector.tensor_scalar(
                            half.bitcast(I8), apts[h][:],
                            SCHR_A, SCHR_B,
                            mybir.AluOpType.mult, mybir.AluOpType.add)

            av_state = {"i": 0}

            def avs(n):
                hs = sb.tile([DH + 1, S], BF16, tag="hid", bufs=3,
                             name=f"hid_{n}")
                for qc in range(2):
                    av = ps.tile([128, 512], F32, tag="small", bufs=2,
                                 name=f"av_{n}_{qc}")
                    for c2 in range(KT // 2):
                        pt = pt_tile(n, c2)
                        ptm = pt[:].rearrange("p (two q) -> p two q", two=2)
                        nc.tensor.matmul(
                            av[0:PVW, :],
                            vp3[:, 2 * c2:2 * c2 + 2,
                                n * PVW:(n + 1) * PVW],
                            ptm[:, :, qc * 512:(qc + 1) * 512],
                            start=(c2 == 0), stop=(c2 == KT // 2 - 1),
                            perf_mode=DR)
                    i = av_state["i"]
                    av_state["i"] += 1
                    dst = hs[:, qc * 512:(qc + 1) * 512]
                    if i % 2 == 0:
                        nc.scalar.copy(dst, av[0:DH + 1, :])
                    else:
                        nc.vector.tensor_copy(dst, av[0:DH + 1, :])
                    eng = nc.sync if i % 2 == 0 else nc.gpsimd
                    eng.dma_start(
                        hid_d[n * (DH + 1):(n + 1) * (DH + 1),
                              qc * 512:(qc + 1) * 512],
                        dst)

            # ---- emission schedule: alpha mini-blocks with DR fillers ----
            proj_qk("q", 0)
            proj_qk("k", 0)
            proj_qk("q", 1)
            proj_qk("k", 1)
            for k in range(KT):           # t0 alphas, proj_v fillers
                alpha_pair(0, k)
                proj_v(k)
            proj_qk("q", 2)
            proj_qk("k", 2)
            for k in range(KT):           # t1 alphas, av fillers
                alpha_pair(1, k)
                if k == 2:
                    avs(0)
                elif k == 6:
                    avs(1)
            proj_qk("q", 3)
            proj_qk("k", 3)
            for k in range(KT):           # t2 alphas
                alpha_pair(2, k)
                if k == 2:
                    avs(2)
                elif k == 6:
                    avs(3)
            for k in range(KT):           # t3 alphas
                alpha_pair(3, k)
                if k == 2:
                    avs(4)
                elif k == 6:
                    avs(5)
            avs(6)
            avs(7)

    nc.compile()
    return nc


_NC_CACHE = {}


def _get_nc(mode, bias_v=False):
    key = (mode, bias_v)
    if key not in _NC_CACHE:
        _NC_CACHE[key] = build(mode, bias_v)
    return _NC_CACHE[key]


def _prep_inputs(inputs, mode):
    q = np.asarray(inputs["query"], np.float32)
    k = np.asarray(inputs["key"], np.float32)
    v = np.asarray(inputs["value"], np.float32)
    Wq = np.asarray(inputs["Wq"], np.float32)
    Wk = np.asarray(inputs["Wk"], np.float32)
    Wv = np.asarray(inputs["Wv"], np.float32)
    bq = np.asarray(inputs["bq"], np.float32)
    bk = np.asarray(inputs["bk"], np.float32)
    bv = np.asarray(inputs["bv"], np.float32)

    def xprep(x, b):
        # [H, S] -> [p, sc, k, s'] -> [128, 8192] fp8
        xt = np.ascontiguousarray(x[b].T).astype(E4NP)
        return np.ascontiguousarray(
            xt.reshape(KT, 128, 2, 512).transpose(1, 2, 0, 3)
        ).reshape(128, 2 * KT * 512)

    def wqk_prep(W, sl):
        # W[sl].T: [H, GH] -> [p, ot, k, o'] -> [128, 4096] fp8
        wt = np.ascontiguousarray(W[sl, :].T).astype(E4NP)
        return np.ascontiguousarray(
            wt.reshape(KT, 128, OT, 128).transpose(1, 2, 0, 3)
        ).reshape(128, OT * KT * 128)

    def wv_prep(W, sl):
        # W[sl].T: [H, GH] -> [p, k, o] -> [128, 4096] fp8
        wt = np.ascontiguousarray(W[sl, :].T).astype(E4NP)
        return np.ascontiguousarray(
            wt.reshape(KT, 128, GH).transpose(1, 0, 2)
        ).reshape(128, KT * GH)

    xq = [xprep(q, b) for b in range(B)]
    xk = [xprep(k, b) for b in range(B)]
    xv = [xprep(v, b) for b in range(B)]
    in_maps = []
    for c in range(NCORES):
        b, g = c // GROUPS, c % GROUPS
        sl = slice(g * GH, (g + 1) * GH)
        bqk = np.stack([bq[sl].reshape(OT, 128).T,
                        bk[sl].reshape(OT, 128).T], 1).reshape(128, 2 * OT)
        in_maps.append({
            "xq": xq[b], "xk": xk[b], "xv": xv[b],
            "wq": wqk_prep(Wq, sl),
            "wk": wqk_prep(Wk, sl),
            "wv": wv_prep(Wv, sl),
            "bqk": np.ascontiguousarray(bqk, dtype=np.float32),
            "bv": np.ascontiguousarray(bv[None, sl]).astype(E4NP),
        })
    return in_maps


def run(inputs, mode=MODE, trace=False):
    bias_v = bool(np.any(np.asarray(inputs["bv"], np.float32)))
    nc = _get_nc(mode, bias_v)
    in_maps = _prep_inputs(inputs, mode)
    res = bass_utils.run_bass_kernel_spmd(
        nc, in_maps, core_ids=list(range(NCORES)), trace=trace)

    masks = np.asarray(inputs["masks"], np.float32)
    query = np.asarray(inputs["query"], np.float32)
    out = np.empty((B, S, H), np.float32)
    for c in range(NCORES):
        b, g = c // GROUPS, c % GROUPS
        hid = np.asarray(res.results[c]["hid"],
                         dtype=np.float32).reshape(HL, DH + 1, S)
        hT = hid[:, :DH, :]                      # (HL, DH, S)
        se = hid[:, DH, :]                       # (HL, S)
        blk = (hT / se[:, None, :]).transpose(2, 0, 1).reshape(S, GH)
        out[b, :, g * GH:(g + 1) * GH] = blk
    out = out * masks[:, :, None] + query
    return out, res


def kernel(**inputs) -> np.ndarray:
    out, _ = run(inputs)
    return out


# revision 27
# speedup vs baseline: 1.4668x; 1.0135x over previous
"""Multi-head attention (ReLU-gated projections) on 8 Trainium2 NeuronCores.

Problem (hardcoded): B=4, S=1024, H=1024, NH=16, DH=64.
  qp = relu(q @ Wq.T + bq); kp, vp likewise
  alpha = softmax(qh @ kh.T / sqrt(DH)) * mask[q]
  out = (alpha @ vh).reshape(B,S,H) + query

Sharding: 8 cores = 4 batches x 2 head-groups (8 heads / 512 hidden cols each).

fp8 design (per core):
  - inputs x/W quantized host-side to fp8e4m3 (TRN float8e4, max 240).
  - projections as fp8 DoubleRow matmuls (2x contraction per cycle):
    qp/kp evac'd with fused bias+relu to bf16, vp to fp8 (with a ones column
    per head so AV accumulates sumexp for free, plus one pad column so the
    DoubleRow pair stride is 16B-aligned: 66 cols/head).
  - alpha: bf16 K=64 matmuls, two heads concurrently on disjoint 64-row
    PE row-groups (2x row tiling; tile_position auto-derived from
    base_partition of the kp/qp slices).
  - softmax exp with a global shift: pt = exp(alpha*SCALE - SHIFT) so pt fits
    fp8e4m3 range; numerator and denominator scale together so the softmax is
    exact.  exp split across two engines: ACT (spline exp -> fp8 out) and DVE
    (Schraudolph: k8 = (alpha*A + B) as int8, bitcast to fp8e4m3).
  - AV: fp8 DoubleRow (vp pairs of k-chunks stationary, pt pairs moving),
    av PSUM [66,512] DMA'd directly to DRAM (rows 0-64; row 64 = sumexp).
  - host divides by sumexp, applies mask, adds residual.
"""
import sys

sys.path.insert(0, "/opt/trn_rl_repo")

import math
import os
import numpy as np
import ml_dtypes

import concourse.bass as bass
import concourse.tile as tile
from concourse import bacc, mybir
from concourse import bass_utils

if os.environ.get("BASS_LDW_OPT", "0") == "1":
    _orig_run_command = bass_utils.run_command

    def _patched_run_command(cmd, **kw):
        cmd = ["--enable-ldw-opt=true" if c == "--enable-ldw-opt=false" else c
               for c in cmd]
        return _orig_run_command(cmd, **kw)

    bass_utils.run_command = _patched_run_command

B, S, H = 4, 1024, 1024
NH, DH = 16, 64
NCORES = 8
GROUPS = 2          # head-groups (tensor-parallel dim)
HL = NH // GROUPS   # heads per core = 8
GH = H // GROUPS    # hidden cols per core = 512
KT = H // 128       # contraction k-tiles = 8
OT = GH // 128      # output o-tiles per core = 4
SCALE = 1.0 / float(np.sqrt(DH))
SHIFT = 4.0         # global exp shift: pt = exp(alpha*SCALE - SHIFT)
PVW = DH + 2        # padded per-head v width (64 v + 1 ones + 1 pad) = 66
VW = HL * PVW       # v cols per k-chunk = 528 (16B aligned)

# Schraudolph constants for fp8e4m3 (bias 7, 3 mantissa bits):
#   k8 = (alpha * SCALE - SHIFT) * 8/ln2 + 56 - c ; bitcast int8 -> fp8
_LN2 = math.log(2.0)
SCHR_C = float(os.environ.get("BASS_SCHR_C", "0.45"))
SCHR_A = SCALE * 8.0 / _LN2
# +0.5: DVE f32->int8 convert truncates (matches CoreSim); makes it rounding.
SCHR_B = 56.0 - SCHR_C - SHIFT * 8.0 / _LN2 \
    + float(os.environ.get("BASS_SCHR_HALF", "0.5"))

MODE = os.environ.get("BASS_MM_DT", "fp8")
WARM = int(os.environ.get("BASS_WARM", "6"))
ACT_EXPS = int(os.environ.get("BASS_ACT_EXPS", "32"))  # of 64 exp tiles

F32 = mybir.dt.float32
BF16 = mybir.dt.bfloat16
FP8 = mybir.dt.float8e4
I8 = mybir.dt.int8
DR = mybir.MatmulPerfMode.DoubleRow
E4NP = ml_dtypes.float8_e4m3


def build(mode, bias_v=False):
    assert mode == "fp8"
    nc = bacc.Bacc("TRN2", target_bir_lowering=False, debug=False,
                   num_devices=NCORES)

    # x: [p, sc(2), k(8), s'(512)] -> [128, 8192]; h = k*128+p, s = sc*512+s'
    xq_d = nc.dram_tensor("xq", [128, 2 * KT * 512], FP8,
                          kind="ExternalInput").ap()
    xk_d = nc.dram_tensor("xk", [128, 2 * KT * 512], FP8,
                          kind="ExternalInput").ap()
    xv_d = nc.dram_tensor("xv", [128, 2 * KT * 512], FP8,
                          kind="ExternalInput").ap()
    # wq/wk: [p, ot(4), k(8), o'(128)] -> [128, 4096]
    wq_d = nc.dram_tensor("wq", [128, OT * KT * 128], FP8,
                          kind="ExternalInput").ap()
    wk_d = nc.dram_tensor("wk", [128, OT * KT * 128], FP8,
                          kind="ExternalInput").ap()
    # wv: [p, k(8), o(512)] -> [128, 4096]
    wv_d = nc.dram_tensor("wv", [128, KT * GH], FP8, kind="ExternalInput").ap()
    bqk_d = nc.dram_tensor("bqk", [128, 2 * OT], F32, kind="ExternalInput").ap()
    bv_d = nc.dram_tensor("bv", [1, GH], FP8, kind="ExternalInput").ap()
    hid_d = nc.dram_tensor("hid", [HL * (DH + 1), S], BF16,
                           kind="ExternalOutput").ap()

    with tile.TileContext(nc) as tc:
        with tc.tile_pool(name="sb", bufs=1) as sb, \
             tc.tile_pool(name="ps", bufs=1, space="PSUM") as ps:

            # ---- persistent SBUF tiles (one per DMA for fine-grain deps) ----
            HK = KT // 2 * 512  # 2048 cols per (sc, k-half)
            x_t = {}
            for which in ("q", "k", "v"):
                for sc in (0, 1):
                    for kh in (0, 1):
                        nm = f"x{which}{sc}{kh}"
                        x_t[(which, sc, kh)] = sb.tile(
                            [128, HK], FP8, tag=nm, name=nm)
            wq_t = [sb.tile([128, 2 * KT * 128], FP8, tag=f"wq{i}",
                            name=f"wq{i}") for i in range(2)]
            wk_t = [sb.tile([128, 2 * KT * 128], FP8, tag=f"wk{i}",
                            name=f"wk{i}") for i in range(2)]
            wv_t = [sb.tile([128, KT // 2 * GH], FP8, tag=f"wv{i}",
                            name=f"wv{i}") for i in range(2)]
            qp_t = [sb.tile([128, S], BF16, tag=f"qp{t}", name=f"qp{t}")
                    for t in range(OT)]
            kp_t = [sb.tile([128, S], BF16, tag=f"kp{t}", name=f"kp{t}")
                    for t in range(OT)]
            vp_t = sb.tile([128, KT * VW], FP8, tag="vp", name="vp")
            bqk_t = sb.tile([128, 2 * OT], F32, tag="bqk", name="bqk")
            bv_t = sb.tile([1, GH], FP8, tag="bv", name="bv")
            wa_t = sb.tile([128, 128], FP8, tag="wa", name="wa")
            wb_t = sb.tile([128, 512], FP8, tag="wb", name="wb")
            dummy_t = sb.tile([1, 8], F32, tag="dummy", name="dummy")
            nshift_t = sb.tile([128, 1], F32, tag="nshift", name="nshift")

            # ---- t=0: DMA-free warmup (memset inputs) + const setup ----
            nc.vector.memset(wa_t[:], 1.0)
            nc.vector.memset(wb_t[:], 0.0)
            nc.gpsimd.memset(nshift_t[:], -SHIFT)
            # vp ones + pad columns (per head, per k-chunk)
            vp4 = vp_t[:].rearrange("p (k n c) -> p k n c", n=HL, c=PVW)
            nc.vector.memset(vp4[:, :, :, DH:DH + 1], 1.0)
            nc.vector.memset(vp4[:, :, :, DH + 1:DH + 2], 0.0)
            for i in range(WARM):
                warm = ps.tile([128, 512], F32, tag="small", bufs=2,
                               name=f"warm{i}")
                nc.tensor.matmul(warm[:], wa_t[:], wb_t[:],
                                 start=True, stop=True)

            # ---- input DMAs: k-half quarters, priority q > k > v ----
            def xq4(which, sc, kh):
                xd = {"q": xq_d, "k": xk_d, "v": xv_d}[which]
                dst = x_t[(which, sc, kh)][:]
                src = xd[:, sc * KT * 512 + kh * HK:
                         sc * KT * 512 + (kh + 1) * HK]
                return dst, src

            nc.gpsimd.dma_start(bqk_t[:], bqk_d)
            nc.gpsimd.dma_start(bv_t[:], bv_d)
            nc.gpsimd.dma_start(wq_t[0][:], wq_d[:, 0:2 * KT * 128])
            nc.gpsimd.dma_start(wk_t[0][:], wk_d[:, 0:2 * KT * 128])
            nc.gpsimd.dma_start(wq_t[1][:], wq_d[:, 2 * KT * 128:])
            nc.gpsimd.dma_start(wk_t[1][:], wk_d[:, 2 * KT * 128:])
            for which in ("q", "k"):
                for sc in (0, 1):
                    nc.sync.dma_start(*xq4(which, sc, 0))
                    nc.scalar.dma_start(*xq4(which, sc, 1))
            nc.sync.dma_start(*xq4("v", 0, 0))
            nc.scalar.dma_start(*xq4("v", 1, 0))
            nc.gpsimd.dma_start(*xq4("v", 0, 1))
            nc.gpsimd.dma_start(*xq4("v", 1, 1))
            nc.sync.dma_start(wv_t[0][:], wv_d[:, 0:KT // 2 * GH])
            nc.scalar.dma_start(wv_t[1][:], wv_d[:, KT // 2 * GH:])
            # preload ACT exp table (after the scalar-ring DMA kicks)
            nc.scalar.activation(dummy_t[:], wb_t[0:1, 0:8],
                                 mybir.ActivationFunctionType.Exp, scale=1.0)

            # rearranged views: (which, sc, c2) -> moving/stationary k-pair
            def wqk_pair(w_t, ot, c2):
                half = w_t[ot // 2]
                base = (ot % 2) * KT * 128
                return half[:, base:base + KT * 128].rearrange(
                    "p (k o) -> p k o", o=128)[:, 2 * c2:2 * c2 + 2, :]

            def x_pair(which, sc, c2):
                kh, c = c2 // 2, c2 % 2
                return x_t[(which, sc, kh)][:].rearrange(
                    "p (k s) -> p k s", s=512)[:, 2 * c:2 * c + 2, :]

            def wv_pair(c2):
                kh, c = c2 // 2, c2 % 2
                return wv_t[kh][:].rearrange(
                    "p (k o) -> p k o", o=GH)[:, 2 * c:2 * c + 2, :]

            vp3 = vp_t[:].rearrange("p (k m) -> p k m", m=VW)

            # ---- engine balancing for exp tiles ----
            exp_state = {"acc": 0}

            def exp_engine():
                exp_state["acc"] += ACT_EXPS
                if exp_state["acc"] >= 64:
                    exp_state["acc"] -= 64
                    return "act"
                return "dve"

            pt_tiles = {}

            def pt_tile(n, c):
                if (n, c) not in pt_tiles:
                    pt_tiles[(n, c)] = sb.tile([128, 2048], FP8, tag="pt",
                                               bufs=32, name=f"pt_{n}_{c}")
                return pt_tiles[(n, c)]

            # ---- stage helpers ----
            def proj_qk(which, ot):
                w_t = wq_t if which == "q" else wk_t
                dst = qp_t[ot] if which == "q" else kp_t[ot]
                wi = 0 if which == "q" else 1
                bias = bqk_t[:, wi * OT + ot:wi * OT + ot + 1]
                pp = ps.tile([128, 1024], F32, tag="apt", bufs=3,
                             name=f"pp{which}{ot}")
                for sc in range(2):
                    for c2 in range(KT // 2):
                        nc.tensor.matmul(
                            pp[:, sc * 512:(sc + 1) * 512],
                            wqk_pair(w_t, ot, c2),
                            x_pair(which, sc, c2),
                            start=(c2 == 0), stop=(c2 == KT // 2 - 1),
                            perf_mode=DR)
                nc.scalar.activation(
                    dst[:], pp[:],
                    mybir.ActivationFunctionType.Relu,
                    bias=bias, scale=1.0)

            def proj_v(st):
                sc, j = st // 4, st % 4
                pp = ps.tile([128, 512], F32, tag="small", bufs=2,
                             name=f"ppv{st}")
                if bias_v:
                    nc.tensor.matmul(pp[:], wa_t[0:1, :], bv_t[:],
                                     start=True, stop=False)
                for c2 in range(KT // 2):
                    nc.tensor.matmul(
                        pp[:],
                        x_pair("v", sc, c2)[:, :, j * 128:(j + 1) * 128],
                        wv_pair(c2),
                        start=(c2 == 0 and not bias_v),
                        stop=(c2 == KT // 2 - 1),
                        perf_mode=DR)
                # evac with relu into the strided fp8 v layout (cols 0..63)
                vdst = vp4[:, st, :, 0:DH]
                psrc = pp[:].rearrange("p (n c) -> p n c", c=DH)
                nc.vector.tensor_scalar(
                    vdst, psrc, 0.0, None, mybir.AluOpType.max)

            def alpha_pair(t, k):
                """alpha + exp for heads (2t, 2t+1), sk-tile k: two K=64
                matmuls on disjoint PE row-groups run concurrently."""
                apts = []
                for h in range(2):
                    apt = ps.tile([128, 1024], F32, tag="apt", bufs=3,
                                  name=f"alp_{2 * t + h}_{k}")
                    apts.append(apt)
                for h in range(2):
                    for qc in range(2):
                        pr = slice(h * 64, h * 64 + 64)
                        nc.tensor.matmul(
                            apts[h][:, qc * 512:(qc + 1) * 512],
                            kp_t[t][pr, k * 128:(k + 1) * 128],
                            qp_t[t][pr, qc * 512:(qc + 1) * 512],
                            start=True, stop=True)
                for h in range(2):
                    n = 2 * t + h
                    pt = pt_tile(n, k // 2)
                    half = pt[:, (k % 2) * 1024:(k % 2) * 1024 + 1024]
                    if exp_engine() == "act":
                        nc.scalar.activation(
                            half, apts[h][:],
                            mybir.ActivationFunctionType.Exp,
                            bias=nshift_t[:], scale=SCALE)
                    else:
                        nc.vector.tensor_scalar(
                            half.bitcast(I8), apts[h][:],
                            SCHR_A, SCHR_B,
                            mybir.AluOpType.mult, mybir.AluOpType.add)

            av_state = {"i": 0}
            hs_tiles = {}

            def av_qc(n, qc):
                if n not in hs_tiles:
                    hs_tiles[n] = sb.tile([DH + 1, S], BF16, tag="hid",
                                          bufs=3, name=f"hid_{n}")
                hs = hs_tiles[n]
                av = ps.tile([128, 512], F32, tag="small", bufs=2,
                             name=f"av_{n}_{qc}")
                for c2 in range(KT // 2):
                    pt = pt_tile(n, c2)
                    ptm = pt[:].rearrange("p (two q) -> p two q", two=2)
                    nc.tensor.matmul(
                        av[0:PVW, :],
                        vp3[:, 2 * c2:2 * c2 + 2,
                            n * PVW:(n + 1) * PVW],
                        ptm[:, :, qc * 512:(qc + 1) * 512],
                        start=(c2 == 0), stop=(c2 == KT // 2 - 1),
                        perf_mode=DR)
                i = av_state["i"]
                av_state["i"] += 1
                dst = hs[:, qc * 512:(qc + 1) * 512]
                if i % 2 == 0:
                    nc.scalar.copy(dst, av[0:DH + 1, :])
                else:
                    nc.vector.tensor_copy(dst, av[0:DH + 1, :])
                eng = nc.sync if i % 2 == 0 else nc.gpsimd
                eng.dma_start(
                    hid_d[n * (DH + 1):(n + 1) * (DH + 1),
                          qc * 512:(qc + 1) * 512],
                    dst)

            # ---- emission schedule: alpha mini-blocks with DR fillers ----
            proj_qk("q", 0)
            proj_qk("k", 0)
            proj_qk("q", 1)
            proj_qk("k", 1)
            for k in range(KT):           # t0 alphas, proj_v fillers
                alpha_pair(0, k)
                proj_v(k)
            proj_qk("q", 2)
            proj_qk("k", 2)
            for k in range(KT):           # t1 alphas, av fillers
                alpha_pair(1, k)
                if k >= 2 and k <= 5:
                    av_qc(*divmod(k - 2, 2))
            proj_qk("q", 3)
            proj_qk("k", 3)
            for k in range(KT):           # t2 alphas
                alpha_pair(2, k)
                if k >= 1 and k <= 4:
                    av_qc(*divmod(k + 3, 2))
            for k in range(KT):           # t3 alphas
                alpha_pair(3, k)
                if k >= 1 and k <= 4:
                    av_qc(*divmod(k + 7, 2))
            av_qc(6, 0)
            av_qc(6, 1)
            av_qc(7, 0)
            av_qc(7, 1)

    nc.compile()
    return nc


_NC_CACHE = {}


def _get_nc(mode, bias_v=False):
    key = (mode, bias_v)
    if key not in _NC_CACHE:
        _NC_CACHE[key] = build(mode, bias_v)
    return _NC_CACHE[key]


def _prep_inputs(inputs, mode):
    q = np.asarray(inputs["query"], np.float32)
    k = np.asarray(inputs["key"], np.float32)
    v = np.asarray(inputs["value"], np.float32)
    Wq = np.asarray(inputs["Wq"], np.float32)
    Wk = np.asarray(inputs["Wk"], np.float32)
    Wv = np.asarray(inputs["Wv"], np.float32)
    bq = np.asarray(inputs["bq"], np.float32)
    bk = np.asarray(inputs["bk"], np.float32)
    bv = np.asarray(inputs["bv"], np.float32)

    def xprep(x, b):
        # [H, S] -> [p, sc, k, s'] -> [128, 8192] fp8
        xt = np.ascontiguousarray(x[b].T).astype(E4NP)
        return np.ascontiguousarray(
            xt.reshape(KT, 128, 2, 512).transpose(1, 2, 0, 3)
        ).reshape(128, 2 * KT * 512)

    def wqk_prep(W, sl):
        # W[sl].T: [H, GH] -> [p, ot, k, o'] -> [128, 4096] fp8
        wt = np.ascontiguousarray(W[sl, :].T).astype(E4NP)
        return np.ascontiguousarray(
            wt.reshape(KT, 128, OT, 128).transpose(1, 2, 0, 3)
        ).reshape(128, OT * KT * 128)

    def wv_prep(W, sl):
        # W[sl].T: [H, GH] -> [p, k, o] -> [128, 4096] fp8
        wt = np.ascontiguousarray(W[sl, :].T).astype(E4NP)
        return np.ascontiguousarray(
            wt.reshape(KT, 128, GH).transpose(1, 0, 2)
        ).reshape(128, KT * GH)

    xq = [xprep(q, b) for b in range(B)]
    xk = [xprep(k, b) for b in range(B)]
    xv = [xprep(v, b) for b in range(B)]
    in_maps = []
    for c in range(NCORES):
        b, g = c // GROUPS, c % GROUPS
        sl = slice(g * GH, (g + 1) * GH)
        bqk = np.stack([bq[sl].reshape(OT, 128).T,
                        bk[sl].reshape(OT, 128).T], 1).reshape(128, 2 * OT)
        in_maps.append({
            "xq": xq[b], "xk": xk[b], "xv": xv[b],
            "wq": wqk_prep(Wq, sl),
            "wk": wqk_prep(Wk, sl),
            "wv": wv_prep(Wv, sl),
            "bqk": np.ascontiguousarray(bqk, dtype=np.float32),
            "bv": np.ascontiguousarray(bv[None, sl]).astype(E4NP),
        })
    return in_maps


def run(inputs, mode=MODE, trace=False):
    bias_v = bool(np.any(np.asarray(inputs["bv"], np.float32)))
    nc = _get_nc(mode, bias_v)
    in_maps = _prep_inputs(inputs, mode)
    res = bass_utils.run_bass_kernel_spmd(
        nc, in_maps, core_ids=list(range(NCORES)), trace=trace)

    masks = np.asarray(inputs["masks"], np.float32)
    query = np.asarray(inputs["query"], np.float32)
    out = np.empty((B, S, H), np.float32)
    for c in range(NCORES):
        b, g = c // GROUPS, c % GROUPS
        hid = np.asarray(res.results[c]["hid"],
                         dtype=np.float32).reshape(HL, DH + 1, S)
        hT = hid[:, :DH, :]                      # (HL, DH, S)
        se = hid[:, DH, :]                       # (HL, S)
        blk = (hT / se[:, None, :]).transpose(2, 0, 1).reshape(S, GH)
        out[b, :, g * GH:(g + 1) * GH] = blk
    out = out * masks[:, :, None] + query
    return out, res


def kernel(**inputs) -> np.ndarray:
    out, _ = run(inputs)
    return out
